# revision 5
# baseline (speedup 1.0000x reference)
"""Conv3d(32->64, k=3, pad=1) + BatchNorm(training) + LeakyReLU(0.2) on
(2, 32, 96, 96, 35), distributed over 8 TRN2 NeuronCores.

Strategy:
  - Shard H (96 = 8 x 12 rows per core). Halo rows + spatial zero-padding are
    materialized host-side into a per-core tensor xs of shape (2,32,14,98,39)
    (1 zero row each side of the 12-row H shard; W padded 96->98; D padded
    35->39 so that three d-shifted SBUF copies can be loaded by one DMA).
  - Conv as implicit GEMM: contraction K = 96 = C_in(32) x kd(3). The SBUF
    "slab" for one input row holds 3 partition-groups, group j pre-shifted by
    j elements along D.  Then each of the 9 (kh,kw) taps is a single matmul
    whose rhs is a free-dim-shifted window of the slab; kd is folded into the
    contraction.  PSUM accumulates the 9 taps.
  - M = C_out = 64 uses half the PE columns, so two spatial tiles run as a
    column-pair: tile A -> psum[0:64], tile B -> psum[64:128] (distinct PE
    column groups overlap in hardware).
  - W is tiled 96 = 8 x 13 wait: 7x13 + 1x5; a matmul streams 13w x 37d = 481
    columns (the 2 padded d columns per w are computed but never evicted).
  - BatchNorm (training stats): bn_stats on each PSUM tile (f32) -> bn_aggr
    -> per-partition (mean, var) -> converted to (sum, sumsq) -> tiny
    AllReduce over the 8 cores -> scale/shift folded into a single Lrelu
    activation.  The conv bias b cancels exactly in training-mode BN and is
    unused.  Conv values are parked in SBUF as bf16 (stats are computed from
    the f32 PSUM, so only storage is rounded).
"""

import numpy as np

import concourse.bacc as bacc
import concourse.bass as bass
import concourse.tile as tile
from concourse import mybir
from concourse.bass_utils import run_bass_kernel_spmd

N_CORES = 8
B, C_IN, C_OUT = 2, 32, 64
H, W, D = 96, 96, 35
HS = H // N_CORES          # 12 output rows per core
HR = HS + 2                # 14 input rows (halo)
WP, DP = W + 2, D + 4      # padded W / padded D for the host tensor
DS = D + 2                 # 37: d extent of one slab w-column
ROWLEN = WP * DS           # 3626 elements per slab partition per row
WT = 13                    # w-tile width
NTILE = 8                  # 7 full tiles of 13 + one of 5
WLAST = W - 7 * WT         # 5
NFULL = WT * DS            # 481 matmul free size (full tile)
NLAST = WLAST * DS         # 185 (last tile)
EVF = WT * D               # 455 evicted columns per full tile
EVL = WLAST * D            # 175 evicted for last tile
BLK = B * HS               # 24 (b,h) blocks per core
BLKCOLS = 4 * EVF          # 1820 conv-buffer columns per block per half
NREC = BLK * 4             # 96 bn_stats records
CNT_A = float(BLK * 4 * EVF)        # elements per partition, A half (43680)
CNT_B = float(BLK * (3 * EVF + EVL))  # B half (36960)
N_TOT = float(B * H * W * D)        # 645120
EPS = 1e-5
NEG = 0.2

F32 = mybir.dt.float32
BF16 = mybir.dt.bfloat16

_CACHE = {}


def _build():
    nc = bacc.Bacc("TRN2", target_bir_lowering=False, debug=False,
                   num_devices=N_CORES)
    xs = nc.dram_tensor("xs", [B, C_IN, HR, WP, DP], F32, kind="ExternalInput")
    wt = nc.dram_tensor("wt", [3, 3, 96, C_OUT], F32, kind="ExternalInput")
    gm = nc.dram_tensor("gm", [C_OUT], F32, kind="ExternalInput")
    bt = nc.dram_tensor("bt", [C_OUT], F32, kind="ExternalInput")
    ys = nc.dram_tensor("ys", [B, C_OUT, HS, W, D], F32, kind="ExternalOutput")

    xs_ap = xs.ap()
    ys_ap = ys.ap()
    # xs element strides
    s_d = 1
    s_w = DP
    s_r = WP * DP
    s_c = HR * s_r
    s_b = C_IN * s_c

    with tile.TileContext(nc) as tc:
        with tc.tile_pool(name="singles", bufs=1) as singles, \
             tc.tile_pool(name="slab", bufs=5) as slabp, \
             tc.tile_pool(name="psum", bufs=4, space="PSUM") as psump, \
             tc.tile_pool(name="stg", bufs=3) as stgp, \
             tc.tile_pool(name="dram", bufs=1, space="DRAM") as dramp:

            # ---- one-time loads ----
            wtile = singles.tile([96, 9, C_OUT], F32)
            nc.sync.dma_start(
                out=wtile,
                in_=wt.ap().rearrange("kh kw p o -> p (kh kw) o"))
            gmt = singles.tile([C_OUT, 1], F32)
            nc.sync.dma_start(out=gmt, in_=gm.ap().rearrange("(p o) -> p o", o=1))
            btt = singles.tile([C_OUT, 1], F32)
            nc.sync.dma_start(out=btt, in_=bt.ap().rearrange("(p o) -> p o", o=1))

            cb = singles.tile([128, BLK * BLKCOLS], BF16)   # conv results
            st = singles.tile([128, NREC * 6], F32)          # bn_stats records

            # ---- pass 1: conv + stats ----
            for b in range(B):
                rows = {}

                def load_row(r, b=b, rows=None):
                    pass

                for h in range(HS):
                    for r in (h, h + 1, h + 2):
                        if r not in rows:
                            rt = slabp.tile([96, ROWLEN], F32, tag="slab")
                            for j in range(3):
                                nc.sync.dma_start(
                                    out=rt[32 * j:32 * (j + 1), :].rearrange(
                                        "p (w d) -> p w d", d=DS),
                                    in_=xs_ap[b, :, r, :, j:j + DS])
                            rows[r] = rt
                    blk = b * HS + h
                    for k in range(4):
                        w0a = 2 * k * WT
                        w0b = (2 * k + 1) * WT
                        nb = NFULL if k < 3 else NLAST
                        ps = psump.tile([128, NFULL], F32, tag="ps")
                        for kh in range(3):
                            rt = rows[h + kh]
                            for kw in range(3):
                                q = kh * 3 + kw
                                first, last = q == 0, q == 8
                                nc.tensor.matmul(
                                    ps[0:64, 0:NFULL],
                                    lhsT=wtile[:, q, :],
                                    rhs=rt[:, (w0a + kw) * DS:(w0a + kw) * DS + NFULL],
                                    start=first, stop=last)
                                nc.tensor.matmul(
                                    ps[64:128, 0:nb],
                                    lhsT=wtile[:, q, :],
                                    rhs=rt[:, (w0b + kw) * DS:(w0b + kw) * DS + nb],
                                    start=first, stop=last)
                        # evict + stats
                        col = blk * BLKCOLS + k * EVF
                        rec = (blk * 4 + k) * 6
                        if k < 3:
                            pv = ps.rearrange("p (w d) -> p w d", d=DS)[:, :, 0:D]
                            nc.scalar.copy(
                                out=cb[:, col:col + EVF].rearrange(
                                    "p (w d) -> p w d", d=D),
                                in_=pv)
                            nc.vector.bn_stats(out=st[:, rec:rec + 6],
                                               in_=cb[:, col:col + EVF])
                        else:
                            pva = ps[0:64, :].rearrange(
                                "p (w d) -> p w d", d=DS)[:, :, 0:D]
                            nc.scalar.copy(
                                out=cb[0:64, col:col + EVF].rearrange(
                                    "p (w d) -> p w d", d=D),
                                in_=pva)
                            nc.vector.bn_stats(out=st[0:64, rec:rec + 6],
                                               in_=cb[0:64, col:col + EVF])
                            pvb = ps[64:128, 0:NLAST].rearrange(
                                "p (w d) -> p w d", d=DS)[:, :, 0:D]
                            nc.scalar.copy(
                                out=cb[64:128, col:col + EVL].rearrange(
                                    "p (w d) -> p w d", d=D),
                                in_=pvb)
                            nc.vector.bn_stats(out=st[64:128, rec:rec + 6],
                                               in_=cb[64:128, col:col + EVL])

            # ---- stats aggregation + allreduce ----
            mv = singles.tile([128, 2], F32)
            nc.vector.bn_aggr(out=mv, in_=st.rearrange("p (r s) -> p r s", s=6))
            npt = singles.tile([128, 1], F32)
            nc.vector.memset(npt[0:64, :], CNT_A)
            nc.vector.memset(npt[64:128, :], CNT_B)
            sq = singles.tile([128, 2], F32)
            t1 = singles.tile([128, 1], F32)
            # sum = mean * n
            nc.vector.tensor_mul(sq[:, 0:1], mv[:, 0:1], npt)
            # sumsq = (var + mean^2) * n
            nc.vector.tensor_mul(t1, mv[:, 0:1], mv[:, 0:1])
            nc.vector.tensor_add(t1, t1, mv[:, 1:2])
            nc.vector.tensor_mul(sq[:, 1:2], t1, npt)

            cc_in = dramp.tile([128, 2], F32)
            cc_out = dramp.tile([128, 2], F32)
            nc.sync.dma_start(out=cc_in[:, :], in_=sq)
            nc.gpsimd.collective_compute(
                "AllReduce", mybir.AluOpType.add,
                replica_groups=[list(range(N_CORES))],
                ins=[cc_in[:, :].opt()], outs=[cc_out[:, :].opt()])
            gl = singles.tile([128, 2], F32)
            nc.sync.dma_start(out=gl, in_=cc_out[:, :])

            hi = singles.tile([64, 2], F32)
            nc.sync.dma_start(out=hi, in_=gl[64:128, :])
            tot = singles.tile([64, 2], F32)
            nc.vector.tensor_add(tot, gl[0:64, :], hi)

            m_g = singles.tile([64, 1], F32)
            qn = singles.tile([64, 1], F32)
            var = singles.tile([64, 1], F32)
            sd = singles.tile([64, 1], F32)
            s64 = singles.tile([64, 1], F32)
            t64 = singles.tile([64, 1], F32)
            nc.vector.tensor_scalar_mul(m_g, tot[:, 0:1], 1.0 / N_TOT)
            nc.vector.tensor_scalar_mul(qn, tot[:, 1:2], 1.0 / N_TOT)
            nc.vector.tensor_mul(var, m_g, m_g)
            nc.vector.tensor_sub(var, qn, var)
            epst = singles.tile([64, 1], F32)
            nc.vector.memset(epst, EPS)
            nc.scalar.activation(out=sd, in_=var,
                                 func=mybir.ActivationFunctionType.Sqrt,
                                 bias=epst)
            nc.vector.reciprocal(out=sd, in_=sd)
            nc.vector.tensor_mul(s64, sd, gmt)      # s = gamma * rsqrt(var+eps)
            nc.vector.tensor_mul(t64, m_g, s64)
            nc.vector.tensor_sub(t64, btt, t64)     # t = beta - mean * s

            s_all = singles.tile([128, 1], F32)
            t_all = singles.tile([128, 1], F32)
            nc.vector.tensor_copy(s_all[0:64, :], s64)
            nc.vector.tensor_copy(t_all[0:64, :], t64)
            nc.sync.dma_start(out=s_all[64:128, :], in_=s_all[0:64, :])
            nc.sync.dma_start(out=t_all[64:128, :], in_=t_all[0:64, :])

            # ---- pass 2: normalize + LeakyReLU + writeback ----
            c_step = HS * W * D  # ys channel stride
            for blk in range(BLK):
                b_, h_ = divmod(blk, HS)
                stg = stgp.tile([128, BLKCOLS], F32, tag="stg")
                nc.scalar.activation(
                    out=stg, in_=cb[:, blk * BLKCOLS:(blk + 1) * BLKCOLS],
                    func=mybir.ActivationFunctionType.Prelu,
                    bias=t_all, scale=s_all, alpha=NEG)
                base_off = ys_ap.offset + b_ * (C_OUT * c_step) + h_ * (W * D)
                # A half: w-tiles 0,2,4,6 at w = 0,26,52,78
                dst_a = bass.AP(
                    tensor=ys_ap.tensor, offset=base_off,
                    ap=[[c_step, C_OUT], [2 * WT * D, 4], [D, WT], [1, D]])
                nc.sync.dma_start(
                    out=dst_a,
                    in_=stg[0:64, :].rearrange("p (t w d) -> p t w d", t=4, d=D))
                # B half: w-tiles 1,3,5 at w = 13,39,65
                dst_b = bass.AP(
                    tensor=ys_ap.tensor, offset=base_off + WT * D,
                    ap=[[c_step, C_OUT], [2 * WT * D, 3], [D, WT], [1, D]])
                nc.sync.dma_start(
                    out=dst_b,
                    in_=stg[64:128, 0:3 * EVF].rearrange(
                        "p (t w d) -> p t w d", t=3, d=D))
                # B last: w-tile 7 at w = 91 (width 5)
                dst_l = bass.AP(
                    tensor=ys_ap.tensor, offset=base_off + 7 * WT * D,
                    ap=[[c_step, C_OUT], [D, WLAST], [1, D]])
                nc.sync.dma_start(
                    out=dst_l,
                    in_=stg[64:128, 3 * EVF:3 * EVF + EVL].rearrange(
                        "p (w d) -> p w d", d=D))

    nc.finalize()
    return nc


def _get_nc():
    if "nc" not in _CACHE:
        _CACHE["nc"] = _build()
    return _CACHE["nc"]


def _prep(x, w, gamma, beta):
    xpad = np.zeros((B, C_IN, H + 2, WP, DP), dtype=np.float32)
    xpad[:, :, 1:H + 1, 1:W + 1, 1:D + 1] = x
    wt = np.ascontiguousarray(
        np.asarray(w, dtype=np.float32).transpose(2, 3, 4, 1, 0).reshape(3, 3, 96, C_OUT))
    gm = np.ascontiguousarray(np.asarray(gamma, dtype=np.float32))
    bt = np.ascontiguousarray(np.asarray(beta, dtype=np.float32))
    in_maps = []
    for c in range(N_CORES):
        xs = np.ascontiguousarray(xpad[:, :, c * HS:c * HS + HR, :, :])
        in_maps.append({"xs": xs, "wt": wt, "gm": gm, "bt": bt})
    return in_maps


def kernel(x, w, b, gamma, beta):
    nc = _get_nc()
    in_maps = _prep(np.asarray(x, dtype=np.float32), w, gamma, beta)
    res = run_bass_kernel_spmd(nc, in_maps, core_ids=list(range(N_CORES)))
    out = np.concatenate([res.results[c]["ys"] for c in range(N_CORES)], axis=2)
    return out.astype(np.float32)


# revision 6
# speedup vs baseline: 1.7557x; 1.7557x over previous
"""Conv3d(32->64, k=3, pad=1) + BatchNorm(training) + LeakyReLU(0.2) on
(2, 32, 96, 96, 35), distributed over 8 TRN2 NeuronCores.

Strategy:
  - Shard H (96 = 8 x 12 rows per core). Halo rows + spatial zero-padding are
    materialized host-side into a per-core tensor xs of shape (2,32,14,98,39)
    (1 zero row each side of the 12-row H shard; W padded 96->98; D padded
    35->39 so that three d-shifted SBUF copies can be loaded by one DMA).
  - Conv as implicit GEMM: contraction K = 96 = C_in(32) x kd(3). The SBUF
    "slab" for one input row holds 3 partition-groups, group j pre-shifted by
    j elements along D.  Then each of the 9 (kh,kw) taps is a single matmul
    whose rhs is a free-dim-shifted window of the slab; kd is folded into the
    contraction.  PSUM accumulates the 9 taps.
  - M = C_out = 64 uses half the PE columns, so two spatial tiles run as a
    column-pair: tile A -> psum[0:64], tile B -> psum[64:128] (distinct PE
    column groups overlap in hardware).
  - W is tiled 96 = 8 x 13 wait: 7x13 + 1x5; a matmul streams 13w x 37d = 481
    columns (the 2 padded d columns per w are computed but never evicted).
  - BatchNorm (training stats): bn_stats on each PSUM tile (f32) -> bn_aggr
    -> per-partition (mean, var) -> converted to (sum, sumsq) -> tiny
    AllReduce over the 8 cores -> scale/shift folded into a single Lrelu
    activation.  The conv bias b cancels exactly in training-mode BN and is
    unused.  Conv values are parked in SBUF as bf16 (stats are computed from
    the f32 PSUM, so only storage is rounded).
"""

import numpy as np

import concourse.bacc as bacc
import concourse.bass as bass
import concourse.tile as tile
from concourse import mybir
from concourse.bass_utils import run_bass_kernel_spmd

N_CORES = 8
B, C_IN, C_OUT = 2, 32, 64
H, W, D = 96, 96, 35
HS = H // N_CORES          # 12 output rows per core
HR = HS + 2                # 14 input rows (halo)
WP, DP = W + 2, D + 4      # padded W / padded D for the host tensor
DS = D + 2                 # 37: d extent of one slab w-column
ROWLEN = WP * DS           # 3626 elements per slab partition per row
WT = 13                    # w-tile width
NTILE = 8                  # 7 full tiles of 13 + one of 5
WLAST = W - 7 * WT         # 5
NFULL = WT * DS            # 481 matmul free size (full tile)
NLAST = WLAST * DS         # 185 (last tile)
EVF = WT * D               # 455 evicted columns per full tile
EVL = WLAST * D            # 175 evicted for last tile
BLK = B * HS               # 24 (b,h) blocks per core
BLKCOLS = 4 * EVF          # 1820 conv-buffer columns per block per half
NREC = BLK * 4             # 96 bn_stats records
CNT_A = float(BLK * 4 * EVF)        # elements per partition, A half (43680)
CNT_B = float(BLK * (3 * EVF + EVL))  # B half (36960)
N_TOT = float(B * H * W * D)        # 645120
EPS = 1e-5
NEG = 0.2

F32 = mybir.dt.float32
BF16 = mybir.dt.bfloat16
import ml_dtypes
NP_BF16 = ml_dtypes.bfloat16

_CACHE = {}


def _build():
    nc = bacc.Bacc("TRN2", target_bir_lowering=False, debug=False,
                   num_devices=N_CORES)
    xs = nc.dram_tensor("xs", [B, C_IN, HR, WP, DP], BF16, kind="ExternalInput")
    wt = nc.dram_tensor("wt", [3, 3, 96, C_OUT], BF16, kind="ExternalInput")
    gm = nc.dram_tensor("gm", [C_OUT], F32, kind="ExternalInput")
    bt = nc.dram_tensor("bt", [C_OUT], F32, kind="ExternalInput")
    ys = nc.dram_tensor("ys", [B, C_OUT, HS, W, D], F32, kind="ExternalOutput")

    xs_ap = xs.ap()
    ys_ap = ys.ap()
    # xs element strides
    s_d = 1
    s_w = DP
    s_r = WP * DP
    s_c = HR * s_r
    s_b = C_IN * s_c

    with tile.TileContext(nc) as tc:
        with tc.tile_pool(name="singles", bufs=1) as singles, \
             tc.tile_pool(name="slab", bufs=5) as slabp, \
             tc.tile_pool(name="psum", bufs=4, space="PSUM") as psump, \
             tc.tile_pool(name="stg", bufs=3) as stgp, \
             tc.tile_pool(name="dram", bufs=1, space="DRAM") as dramp:

            # ---- one-time loads ----
            wtile = singles.tile([96, 9, C_OUT], BF16)
            nc.sync.dma_start(
                out=wtile,
                in_=wt.ap().rearrange("kh kw p o -> p (kh kw) o"))
            gmt = singles.tile([C_OUT, 1], F32)
            nc.sync.dma_start(out=gmt, in_=gm.ap().rearrange("(p o) -> p o", o=1))
            btt = singles.tile([C_OUT, 1], F32)
            nc.sync.dma_start(out=btt, in_=bt.ap().rearrange("(p o) -> p o", o=1))

            cb = singles.tile([128, BLK * BLKCOLS], BF16)   # conv results
            st = singles.tile([128, NREC * 6], F32)          # bn_stats records

            # ---- pass 1: conv + stats ----
            for b in range(B):
                rows = {}

                def load_row(r, b=b, rows=None):
                    pass

                for h in range(HS):
                    for r in (h, h + 1, h + 2):
                        if r not in rows:
                            rt = slabp.tile([96, ROWLEN], BF16, tag="slab")
                            for j in range(3):
                                nc.sync.dma_start(
                                    out=rt[32 * j:32 * (j + 1), :].rearrange(
                                        "p (w d) -> p w d", d=DS),
                                    in_=xs_ap[b, :, r, :, j:j + DS])
                            rows[r] = rt
                    blk = b * HS + h
                    for k in range(4):
                        w0a = 2 * k * WT
                        w0b = (2 * k + 1) * WT
                        nb = NFULL if k < 3 else NLAST
                        ps = psump.tile([128, NFULL], F32, tag="ps")
                        for kh in range(3):
                            rt = rows[h + kh]
                            for kw in range(3):
                                q = kh * 3 + kw
                                first, last = q == 0, q == 8
                                nc.tensor.matmul(
                                    ps[0:64, 0:NFULL],
                                    lhsT=wtile[:, q, :],
                                    rhs=rt[:, (w0a + kw) * DS:(w0a + kw) * DS + NFULL],
                                    start=first, stop=last)
                                nc.tensor.matmul(
                                    ps[64:128, 0:nb],
                                    lhsT=wtile[:, q, :],
                                    rhs=rt[:, (w0b + kw) * DS:(w0b + kw) * DS + nb],
                                    start=first, stop=last)
                        # evict + stats
                        col = blk * BLKCOLS + k * EVF
                        rec = (blk * 4 + k) * 6
                        if k < 3:
                            pv = ps.rearrange("p (w d) -> p w d", d=DS)[:, :, 0:D]
                            nc.scalar.copy(
                                out=cb[:, col:col + EVF].rearrange(
                                    "p (w d) -> p w d", d=D),
                                in_=pv)
                            nc.vector.bn_stats(out=st[:, rec:rec + 6],
                                               in_=cb[:, col:col + EVF])
                        else:
                            pva = ps[0:64, :].rearrange(
                                "p (w d) -> p w d", d=DS)[:, :, 0:D]
                            nc.scalar.copy(
                                out=cb[0:64, col:col + EVF].rearrange(
                                    "p (w d) -> p w d", d=D),
                                in_=pva)
                            nc.vector.bn_stats(out=st[0:64, rec:rec + 6],
                                               in_=cb[0:64, col:col + EVF])
                            pvb = ps[64:128, 0:NLAST].rearrange(
                                "p (w d) -> p w d", d=DS)[:, :, 0:D]
                            nc.scalar.copy(
                                out=cb[64:128, col:col + EVL].rearrange(
                                    "p (w d) -> p w d", d=D),
                                in_=pvb)
                            nc.vector.bn_stats(out=st[64:128, rec:rec + 6],
                                               in_=cb[64:128, col:col + EVL])

            # ---- stats aggregation + allreduce ----
            mv = singles.tile([128, 2], F32)
            nc.vector.bn_aggr(out=mv, in_=st.rearrange("p (r s) -> p r s", s=6))
            npt = singles.tile([128, 1], F32)
            nc.vector.memset(npt[0:64, :], CNT_A)
            nc.vector.memset(npt[64:128, :], CNT_B)
            sq = singles.tile([128, 2], F32)
            t1 = singles.tile([128, 1], F32)
            # sum = mean * n
            nc.vector.tensor_mul(sq[:, 0:1], mv[:, 0:1], npt)
            # sumsq = (var + mean^2) * n
            nc.vector.tensor_mul(t1, mv[:, 0:1], mv[:, 0:1])
            nc.vector.tensor_add(t1, t1, mv[:, 1:2])
            nc.vector.tensor_mul(sq[:, 1:2], t1, npt)

            cc_in = dramp.tile([128, 2], F32)
            cc_out = dramp.tile([128, 2], F32)
            nc.sync.dma_start(out=cc_in[:, :], in_=sq)
            nc.gpsimd.collective_compute(
                "AllReduce", mybir.AluOpType.add,
                replica_groups=[list(range(N_CORES))],
                ins=[cc_in[:, :].opt()], outs=[cc_out[:, :].opt()])
            gl = singles.tile([128, 2], F32)
            nc.sync.dma_start(out=gl, in_=cc_out[:, :])

            hi = singles.tile([64, 2], F32)
            nc.sync.dma_start(out=hi, in_=gl[64:128, :])
            tot = singles.tile([64, 2], F32)
            nc.vector.tensor_add(tot, gl[0:64, :], hi)

            m_g = singles.tile([64, 1], F32)
            qn = singles.tile([64, 1], F32)
            var = singles.tile([64, 1], F32)
            sd = singles.tile([64, 1], F32)
            s64 = singles.tile([64, 1], F32)
            t64 = singles.tile([64, 1], F32)
            nc.vector.tensor_scalar_mul(m_g, tot[:, 0:1], 1.0 / N_TOT)
            nc.vector.tensor_scalar_mul(qn, tot[:, 1:2], 1.0 / N_TOT)
            nc.vector.tensor_mul(var, m_g, m_g)
            nc.vector.tensor_sub(var, qn, var)
            epst = singles.tile([64, 1], F32)
            nc.vector.memset(epst, EPS)
            nc.scalar.activation(out=sd, in_=var,
                                 func=mybir.ActivationFunctionType.Sqrt,
                                 bias=epst)
            nc.vector.reciprocal(out=sd, in_=sd)
            nc.vector.tensor_mul(s64, sd, gmt)      # s = gamma * rsqrt(var+eps)
            nc.vector.tensor_mul(t64, m_g, s64)
            nc.vector.tensor_sub(t64, btt, t64)     # t = beta - mean * s

            s_all = singles.tile([128, 1], F32)
            t_all = singles.tile([128, 1], F32)
            nc.vector.tensor_copy(s_all[0:64, :], s64)
            nc.vector.tensor_copy(t_all[0:64, :], t64)
            nc.sync.dma_start(out=s_all[64:128, :], in_=s_all[0:64, :])
            nc.sync.dma_start(out=t_all[64:128, :], in_=t_all[0:64, :])

            # ---- pass 2: normalize + LeakyReLU + writeback ----
            c_step = HS * W * D  # ys channel stride
            for blk in range(BLK):
                b_, h_ = divmod(blk, HS)
                stg = stgp.tile([128, BLKCOLS], F32, tag="stg")
                nc.scalar.activation(
                    out=stg, in_=cb[:, blk * BLKCOLS:(blk + 1) * BLKCOLS],
                    func=mybir.ActivationFunctionType.Prelu,
                    bias=t_all, scale=s_all, alpha=NEG)
                base_off = ys_ap.offset + b_ * (C_OUT * c_step) + h_ * (W * D)
                # A half: w-tiles 0,2,4,6 at w = 0,26,52,78
                dst_a = bass.AP(
                    tensor=ys_ap.tensor, offset=base_off,
                    ap=[[c_step, C_OUT], [2 * WT * D, 4], [D, WT], [1, D]])
                nc.sync.dma_start(
                    out=dst_a,
                    in_=stg[0:64, :].rearrange("p (t w d) -> p t w d", t=4, d=D))
                # B half: w-tiles 1,3,5 at w = 13,39,65
                dst_b = bass.AP(
                    tensor=ys_ap.tensor, offset=base_off + WT * D,
                    ap=[[c_step, C_OUT], [2 * WT * D, 3], [D, WT], [1, D]])
                nc.sync.dma_start(
                    out=dst_b,
                    in_=stg[64:128, 0:3 * EVF].rearrange(
                        "p (t w d) -> p t w d", t=3, d=D))
                # B last: w-tile 7 at w = 91 (width 5)
                dst_l = bass.AP(
                    tensor=ys_ap.tensor, offset=base_off + 7 * WT * D,
                    ap=[[c_step, C_OUT], [D, WLAST], [1, D]])
                nc.sync.dma_start(
                    out=dst_l,
                    in_=stg[64:128, 3 * EVF:3 * EVF + EVL].rearrange(
                        "p (w d) -> p w d", d=D))

    nc.finalize()
    return nc


def _get_nc():
    if "nc" not in _CACHE:
        _CACHE["nc"] = _build()
    return _CACHE["nc"]


def _prep(x, w, gamma, beta):
    xpad = np.zeros((B, C_IN, H + 2, WP, DP), dtype=np.float32)
    xpad[:, :, 1:H + 1, 1:W + 1, 1:D + 1] = x
    wt = np.ascontiguousarray(
        np.asarray(w, dtype=np.float32).transpose(2, 3, 4, 1, 0).reshape(
            3, 3, 96, C_OUT)).astype(NP_BF16)
    gm = np.ascontiguousarray(np.asarray(gamma, dtype=np.float32))
    bt = np.ascontiguousarray(np.asarray(beta, dtype=np.float32))
    in_maps = []
    for c in range(N_CORES):
        xs = np.ascontiguousarray(
            xpad[:, :, c * HS:c * HS + HR, :, :]).astype(NP_BF16)
        in_maps.append({"xs": xs, "wt": wt, "gm": gm, "bt": bt})
    return in_maps


def kernel(x, w, b, gamma, beta):
    nc = _get_nc()
    in_maps = _prep(np.asarray(x, dtype=np.float32), w, gamma, beta)
    res = run_bass_kernel_spmd(nc, in_maps, core_ids=list(range(N_CORES)))
    out = np.concatenate([res.results[c]["ys"] for c in range(N_CORES)], axis=2)
    return out.astype(np.float32)
